# revision 3
# baseline (speedup 1.0000x reference)
"""Trainium2 Bass kernel for nn_BatchGraphEncoder (gnn_message_passing).

Math note: the reference's segment softmax uses B unique segment ids
(groups of size 1), so alpha == exp(x-x)/1 == 1.0 bit-exactly for any
finite scores.  The output is therefore independent of the attention
inputs (w_i, w_j, w_k) and reduces to pure batch sums:

    out[:,   0:128] = sum_b h[b,:]      (broadcast over the N=512 rows)
    out[:, 128:256] = sum_b r[b,:]      (broadcast)
    out[:, 256:384] = sum_b t[b,:,:]    ([512, 128])

Memory-bound reduction over B=2048 dominated by reading t (512 MB).
Shard B across the 8 cores, reduce on-device, host-sum the partials.

Per-core the reduction is split across TWO engines so neither ever
paces the ~387 GB/s DMA stream (fp32 tensor_tensor runs at 1x mode,
~1.04 ns/elem — DVE alone sits at ~85% busy and its fold backlog used
to stall the stream's tail for ~15 us):

  * PE path (rows [0,128)): batch-major tiles [128 rows, 8192 cols]
    (32 KB descriptors).  A stationary ones-column matrix sums the 128
    partition rows of each [128,512] block into PSUM row q (the block's
    column index) — fp32 matmul at ~307 GB/s, ~60% busy on its share.
  * DVE path (rows [128,264)): column-major tiles, partition p holds
    flat columns [512p, 512p+512) (2KB descriptors); halving folds into
    a width-1024 accumulator, ~45% busy on its share.

Both paths produce per-column partial sums in the SAME mapping
(partition/psum-row q owns cols [512q, 512q+512)), combined by one DVE
add at the end.  The h/r sums ride the same stationary-ones trick in a
separate PSUM bank, fed before the t matmul group so the two PE
accumulation groups never interleave.

Load balancing: cores 4 and 6 of this machine are chronically ~10%
down on DMA bandwidth, and traces show one further core (currently 2)
pinned down by external HBM traffic for hours at a time, while
sporadically-hit cores drift run to run.  Hedge: cores {2, 4} get 242
rows, core 6 gets 244, the five fast cores keep 264 — a clean day's
max is unchanged and a slow-{2,4,6} day equalizes.  Rows [242, 244)
are skipped on {2, 4}; rows [244, 264) on {2, 4, 6} (their buffers
hold zero padding there).  The conditional tiles sit mid-stream in the
DVE path; their accumulator merge is gated by a per-partition scalar
mask so skipped-DMA garbage never reaches the sums, and h/r padding
rows are zeros, which is exact for a sum.
"""

import numpy as np

B, N, D = 2048, 512, 128
NCORES = 8
FLAT = N * D                 # 65536 flattened (n, d) columns
MMW = 512                    # columns per partition / fold unit
R_PE = 128                   # rows [0, R_PE) reduced on the TensorEngine

B_FAST = 264
SIZES = [B_FAST] * NCORES
SIZES[2] = 242
SIZES[4] = 242
SIZES[6] = 244
assert sum(SIZES) == B
assert R_PE <= min(SIZES), "PE path must only touch rows valid on every core"

# Emission plan. ("pe", r0, nr, j): PE-path batch-major tile — rows
# [r0, r0+nr) x cols [8192j, 8192j+8192).  ("dve", row0, NB, cond):
# DVE-path column-major tile.  DVE and PE tiles interleave 1:1 so
# neither consumer's work bunches at the end of the stream; the heavy
# 16-row DVE folds sit early/mid-stream, the last PE tile lands ~75%
# through, and the plan ends with small DVE tiles whose folds drain
# inside the DMA stream.
TILE_PLAN = [
    ("dve", 128, 16, None),
    ("pe", 0, 128, 0),
    ("dve", 144, 16, None),
    ("pe", 0, 128, 1),
    ("dve", 160, 16, None),
    ("pe", 0, 128, 2),
    ("dve", 176, 16, None),
    ("pe", 0, 128, 3),
    ("dve", 244, 16, "c246"),
    ("pe", 0, 128, 4),
    ("dve", 192, 16, None),
    ("pe", 0, 128, 5),
    ("dve", 208, 16, None),
    ("pe", 0, 128, 6),
    ("dve", 224, 8, None),
    ("pe", 0, 128, 7),
    ("dve", 260, 4, "c246"),
    ("dve", 242, 2, "c24"),
    ("dve", 232, 4, None),
    ("dve", 236, 4, None),
    ("dve", 240, 2, None),
]
_dve_rows = sorted(
    r for it in TILE_PLAN if it[0] == "dve" for r in range(it[1], it[1] + it[2])
)
assert _dve_rows == list(range(R_PE, B_FAST)), "DVE path must cover rows [128,264)"
_pe_cover = sorted(it[3] for it in TILE_PLAN if it[0] == "pe")
assert _pe_cover == list(range(8))

_BUILT = None
# test.py can inject {"trace": True, ...} here; harness path leaves it empty.
RUN_KWARGS = {}
LAST_RESULTS = None


def _build():
    from concourse import bacc, tile, mybir

    f32 = mybir.dt.float32
    add = mybir.AluOpType.add
    nc = bacc.Bacc(
        "TRN2",
        target_bir_lowering=False,
        debug=False,
        enable_asserts=False,
        num_devices=NCORES,
    )
    t_in = nc.dram_tensor("t_shard", [B_FAST, FLAT], f32, kind="ExternalInput").ap()
    h_in = nc.dram_tensor("h_shard", [B_FAST, D], f32, kind="ExternalInput").ap()
    r_in = nc.dram_tensor("r_shard", [B_FAST, D], f32, kind="ExternalInput").ap()
    out_t = nc.dram_tensor("out_t_part", [128, MMW], f32, kind="ExternalOutput").ap()
    out_hr = nc.dram_tensor("out_hr_part", [2, D], f32, kind="ExternalOutput").ap()

    with tile.TileContext(nc) as tc:
        with (
            tc.tile_pool(name="wconst", bufs=1) as wpool,
            tc.tile_pool(name="loads", bufs=6) as loads,
            tc.tile_pool(name="hr", bufs=6) as hrpool,
            tc.tile_pool(name="res", bufs=1) as res,
            tc.tile_pool(name="acc", bufs=2, space="PSUM") as ppool,
        ):
            W = wpool.tile([128, 256], f32)
            maskA = wpool.tile([128, 1], f32)
            maskB = wpool.tile([128, 1], f32)
            psum_hr = ppool.tile([128, D], f32)
            psum_t = ppool.tile([128, MMW], f32)
            acc = res.tile([128, 1024], f32)
            skip_cond = {}
            masks = {"c24": maskA, "c246": maskB}

            def emit_setup_and_hr():
                # W is zero except column 128 == 1.0; W[:, 128-q : 256-q]
                # is a [128, 128] stationary whose column q is all-ones.
                nc.vector.memset(W[:], 0.0)
                nc.vector.memset(W[:, 128:129], 1.0)
                # maskA/maskB = 0.0 on the core(s) that skip that tier.
                nc.vector.memset(maskA[:], 1.0)
                nc.vector.memset(maskB[:], 1.0)
                pid_vec = nc.vector.partition_id()
                with tc.If(pid_vec == 2):
                    nc.vector.memset(maskA[:], 0.0)
                    nc.vector.memset(maskB[:], 0.0)
                with tc.If(pid_vec == 4):
                    nc.vector.memset(maskA[:], 0.0)
                    nc.vector.memset(maskB[:], 0.0)
                with tc.If(pid_vec == 6):
                    nc.vector.memset(maskB[:], 0.0)
                pid_sync = nc.sync.partition_id()
                pid_act = nc.scalar.partition_id()
                skip_cond["c24"] = {
                    nc.sync: (pid_sync != 2) * (pid_sync != 4),
                    nc.scalar: (pid_act != 2) * (pid_act != 4),
                }
                skip_cond["c246"] = {
                    nc.sync: (pid_sync != 2) * (pid_sync != 4) * (pid_sync != 6),
                    nc.scalar: (pid_act != 2) * (pid_act != 4) * (pid_act != 6),
                }

                # h / r batch sums -> rows 0 / 1 of psum_hr.  Emitted (and
                # executed) before the first PE t matmul so the two PSUM
                # accumulation groups stay sequential on the PE queue.
                chunks = []
                for row, src in ((0, h_in), (1, r_in)):
                    for c0 in range(0, B_FAST, 128):
                        k = min(128, B_FAST - c0)
                        ht = hrpool.tile([128, D], f32)
                        nc.gpsimd.dma_start(ht[:k, :], src[c0 : c0 + k, :])
                        chunks.append((row, ht, k))
                for i, (row, ht, k) in enumerate(chunks):
                    nc.tensor.matmul(
                        psum_hr[:],
                        W[:k, 128 - row : 256 - row],
                        ht[:k, :],
                        start=(i == 0),
                        stop=(i == len(chunks) - 1),
                    )

            n_pe = sum(1 for it in TILE_PLAN if it[0] == "pe")
            pe_done = 0
            for k, item in enumerate(TILE_PLAN):
                if k == 1:
                    emit_setup_and_hr()
                dma = nc.sync if k % 2 == 0 else nc.scalar
                tl = loads.tile([128, 16 * MMW], f32)
                if item[0] == "pe":
                    # Batch-major tile: rows [r0, r0+nr) x cols [8192j, +8192).
                    _, r0, nr, j = item
                    dma.dma_start(
                        tl[:nr, :], t_in[r0 : r0 + nr, 8192 * j : 8192 * (j + 1)]
                    )
                    for q in range(16):
                        qg = 16 * j + q  # global column block -> psum row
                        nc.tensor.matmul(
                            psum_t[:],
                            W[:nr, 128 - qg : 256 - qg],
                            tl[:nr, MMW * q : MMW * (q + 1)],
                            start=(pe_done == 0 and q == 0),
                            stop=(pe_done == n_pe - 1 and q == 15),
                        )
                    pe_done += 1
                    continue
                _, b0, NB, cnd = item
                fw = NB * MMW
                src = t_in[b0 : b0 + NB, :].rearrange("b (p c) -> p b c", p=128)
                dst = tl[:, :fw].rearrange("p (b c) -> p b c", b=NB)
                if cnd:
                    # Skipped on the slow core(s): the slot then holds stale
                    # (finite) data; the masked merge zeroes it.
                    dma.dma_start(dst, src, cond=skip_cond[cnd][dma])
                else:
                    dma.dma_start(dst, src)
                half = fw // 2
                while half >= 1024:
                    nc.vector.tensor_tensor(
                        tl[:, :half], tl[:, :half], tl[:, half : 2 * half], add
                    )
                    half //= 2
                if k == 0:
                    nc.vector.tensor_copy(acc[:], tl[:, :1024])
                elif cnd:
                    # acc = (tile_fold * mask) + acc
                    nc.vector.scalar_tensor_tensor(
                        acc[:],
                        tl[:, :1024],
                        masks[cnd][:],
                        acc[:],
                        mybir.AluOpType.mult,
                        add,
                    )
                else:
                    nc.vector.tensor_tensor(acc[:], acc[:], tl[:, :1024], add)

            res_t = res.tile([128, MMW], f32)
            nc.vector.tensor_tensor(res_t[:], acc[:, :512], acc[:, 512:], add)
            nc.vector.tensor_tensor(res_t[:], res_t[:], psum_t[:], add)
            nc.sync.dma_start(out_t[:], res_t[:])

            res_hr = res.tile([2, D], f32)
            nc.vector.tensor_copy(res_hr[:], psum_hr[0:2, :])
            nc.sync.dma_start(out_hr[:], res_hr[:])

    nc.compile()
    return nc


def _get_built():
    global _BUILT
    if _BUILT is None:
        _BUILT = _build()
    return _BUILT


def kernel(h, r, t, w_i, w_j, w_k):
    global LAST_RESULTS
    from concourse import bass_utils

    nc = _get_built()
    t2 = np.ascontiguousarray(t, dtype=np.float32).reshape(B, FLAT)
    h = np.ascontiguousarray(h, dtype=np.float32)
    r = np.ascontiguousarray(r, dtype=np.float32)

    def pad(a, ncols):
        out = np.zeros((B_FAST, ncols), dtype=np.float32)
        out[: a.shape[0]] = a
        return out

    starts = np.concatenate([[0], np.cumsum(SIZES)])
    in_maps = []
    for c in range(NCORES):
        s, e = int(starts[c]), int(starts[c + 1])
        if e - s == B_FAST:
            in_maps.append({"t_shard": t2[s:e], "h_shard": h[s:e], "r_shard": r[s:e]})
        else:
            in_maps.append(
                {
                    "t_shard": pad(t2[s:e], FLAT),
                    "h_shard": pad(h[s:e], D),
                    "r_shard": pad(r[s:e], D),
                }
            )
    results = bass_utils.run_bass_kernel_spmd(
        nc, in_maps, core_ids=list(range(NCORES)), **RUN_KWARGS
    )
    LAST_RESULTS = results

    sum_t = np.zeros(FLAT, dtype=np.float64)
    sum_h = np.zeros(D, dtype=np.float64)
    sum_r = np.zeros(D, dtype=np.float64)
    for c in range(NCORES):
        sum_t += results.results[c]["out_t_part"].reshape(FLAT)
        sum_h += results.results[c]["out_hr_part"][0]
        sum_r += results.results[c]["out_hr_part"][1]

    out = np.empty((N, 3 * D), dtype=np.float32)
    out[:, 0:D] = sum_h.astype(np.float32)[None, :]
    out[:, D : 2 * D] = sum_r.astype(np.float32)[None, :]
    out[:, 2 * D :] = sum_t.astype(np.float32).reshape(N, D)
    return out


# revision 4
# speedup vs baseline: 1.3869x; 1.3869x over previous
"""Trainium2 Bass kernel for nn_BatchGraphEncoder (gnn_message_passing).

Math note: the reference's segment softmax uses B unique segment ids
(groups of size 1), so alpha == exp(x-x)/1 == 1.0 bit-exactly for any
finite scores.  The output is therefore independent of the attention
inputs (w_i, w_j, w_k) and reduces to pure batch sums:

    out[:,   0:128] = sum_b h[b,:]      (broadcast over the N=512 rows)
    out[:, 128:256] = sum_b r[b,:]      (broadcast)
    out[:, 256:384] = sum_b t[b,:,:]    ([512, 128])

Memory-bound reduction over B=2048 dominated by reading t (512 MB).
Shard B across the 8 cores, reduce on-device, host-sum the partials.

Per-core the reduction is split across TWO engines so neither ever
paces the ~387 GB/s DMA stream (fp32 tensor_tensor runs at 1x mode,
~1.04 ns/elem — DVE alone sits at ~85% busy and its fold backlog used
to stall the stream's tail for ~15 us):

  * PE path (rows [0,128)): batch-major tiles [128 rows, 8192 cols]
    (32 KB descriptors).  A stationary ones-column matrix sums the 128
    partition rows of each [128,512] block into PSUM row q (the block's
    column index) — fp32 matmul at ~307 GB/s, ~60% busy on its share.
  * DVE path (rows [128,264)): column-major tiles, partition p holds
    flat columns [512p, 512p+512) (2KB descriptors); halving folds into
    a width-1024 accumulator, ~45% busy on its share.

Both paths produce per-column partial sums in the SAME mapping
(partition/psum-row q owns cols [512q, 512q+512)), combined by one DVE
add at the end.  The h/r sums ride the same stationary-ones trick in a
separate PSUM bank, fed before the t matmul group so the two PE
accumulation groups never interleave.

Load balancing: cores 4 and 6 of this machine are chronically ~10%
down on DMA bandwidth, and traces show one further core (currently 2)
pinned down by external HBM traffic for hours at a time, while
sporadically-hit cores drift run to run.  Hedge: cores {2, 4} get 242
rows, core 6 gets 244, the five fast cores keep 264 — a clean day's
max is unchanged and a slow-{2,4,6} day equalizes.  Rows [242, 244)
are skipped on {2, 4}; rows [244, 264) on {2, 4, 6} (their buffers
hold zero padding there).  The conditional tiles sit mid-stream in the
DVE path; their accumulator merge is gated by a per-partition scalar
mask so skipped-DMA garbage never reaches the sums, and h/r padding
rows are zeros, which is exact for a sum.
"""

import numpy as np

B, N, D = 2048, 512, 128
NCORES = 8
FLAT = N * D                 # 65536 flattened (n, d) columns
MMW = 512                    # columns per partition / fold unit
R_PE = 128                   # rows [0, R_PE) reduced on the TensorEngine

B_FAST = 264
SIZES = [B_FAST] * NCORES
SIZES[2] = 242
SIZES[4] = 242
SIZES[6] = 244
assert sum(SIZES) == B
assert R_PE <= min(SIZES), "PE path must only touch rows valid on every core"

# Emission plan. ("pe", r0, nr, j): PE-path batch-major tile — rows
# [r0, r0+nr) x cols [8192j, 8192j+8192).  ("dve", row0, NB, cond):
# DVE-path column-major tile.  DVE and PE tiles interleave 1:1 so
# neither consumer's work bunches at the end of the stream; the heavy
# 16-row DVE folds sit early/mid-stream, the last PE tile lands ~75%
# through, and the plan ends with small DVE tiles whose folds drain
# inside the DMA stream.
TILE_PLAN = [
    ("dve", 128, 16, None),
    ("pe", 0, 128, 0),
    ("dve", 144, 16, None),
    ("pe", 0, 128, 1),
    ("dve", 160, 16, None),
    ("pe", 0, 128, 2),
    ("dve", 176, 16, None),
    ("pe", 0, 128, 3),
    ("dve", 244, 16, "c246"),
    ("pe", 0, 128, 4),
    ("dve", 192, 16, None),
    ("pe", 0, 128, 5),
    ("dve", 208, 16, None),
    ("pe", 0, 128, 6),
    ("dve", 224, 8, None),
    ("pe", 0, 128, 7),
    ("dve", 260, 4, "c246"),
    ("dve", 242, 2, "c24"),
    ("dve", 232, 4, None),
    ("dve", 236, 4, None),
    ("dve", 240, 2, None),
]
_dve_rows = sorted(
    r for it in TILE_PLAN if it[0] == "dve" for r in range(it[1], it[1] + it[2])
)
assert _dve_rows == list(range(R_PE, B_FAST)), "DVE path must cover rows [128,264)"
_pe_cover = sorted(it[3] for it in TILE_PLAN if it[0] == "pe")
assert _pe_cover == list(range(8))

_BUILT = None
# test.py can inject {"trace": True, ...} here; harness path leaves it empty.
RUN_KWARGS = {}
LAST_RESULTS = None


def _build():
    from concourse import bacc, tile, mybir

    f32 = mybir.dt.float32
    add = mybir.AluOpType.add
    nc = bacc.Bacc(
        "TRN2",
        target_bir_lowering=False,
        debug=False,
        enable_asserts=False,
        num_devices=NCORES,
    )
    t_in = nc.dram_tensor("t_shard", [B_FAST, FLAT], f32, kind="ExternalInput").ap()
    h_in = nc.dram_tensor("h_shard", [B_FAST, D], f32, kind="ExternalInput").ap()
    r_in = nc.dram_tensor("r_shard", [B_FAST, D], f32, kind="ExternalInput").ap()
    out_t = nc.dram_tensor("out_t_part", [128, MMW], f32, kind="ExternalOutput").ap()
    out_hr = nc.dram_tensor("out_hr_part", [2, D], f32, kind="ExternalOutput").ap()

    with tile.TileContext(nc) as tc:
        with (
            tc.tile_pool(name="wconst", bufs=1) as wpool,
            tc.tile_pool(name="loads", bufs=6) as loads,
            tc.tile_pool(name="hr", bufs=6) as hrpool,
            tc.tile_pool(name="res", bufs=1) as res,
            tc.tile_pool(name="acc", bufs=2, space="PSUM") as ppool,
        ):
            W = wpool.tile([128, 256], f32)
            maskA = wpool.tile([128, 1], f32)
            maskB = wpool.tile([128, 1], f32)
            psum_hr = ppool.tile([128, D], f32)
            psum_t = ppool.tile([128, MMW], f32)
            acc = res.tile([128, 1024], f32)
            skip_cond = {}
            masks = {"c24": maskA, "c246": maskB}

            def emit_setup_and_hr():
                # W is zero except column 128 == 1.0; W[:, 128-q : 256-q]
                # is a [128, 128] stationary whose column q is all-ones.
                nc.vector.memset(W[:], 0.0)
                nc.vector.memset(W[:, 128:129], 1.0)
                # maskA/maskB = 0.0 on the core(s) that skip that tier.
                nc.vector.memset(maskA[:], 1.0)
                nc.vector.memset(maskB[:], 1.0)
                pid_vec = nc.vector.partition_id()
                with tc.If(pid_vec == 2):
                    nc.vector.memset(maskA[:], 0.0)
                    nc.vector.memset(maskB[:], 0.0)
                with tc.If(pid_vec == 4):
                    nc.vector.memset(maskA[:], 0.0)
                    nc.vector.memset(maskB[:], 0.0)
                with tc.If(pid_vec == 6):
                    nc.vector.memset(maskB[:], 0.0)
                pid_sync = nc.sync.partition_id()
                pid_act = nc.scalar.partition_id()
                skip_cond["c24"] = {
                    nc.sync: (pid_sync != 2) * (pid_sync != 4),
                    nc.scalar: (pid_act != 2) * (pid_act != 4),
                }
                skip_cond["c246"] = {
                    nc.sync: (pid_sync != 2) * (pid_sync != 4) * (pid_sync != 6),
                    nc.scalar: (pid_act != 2) * (pid_act != 4) * (pid_act != 6),
                }

                # h / r batch sums -> rows 0 / 1 of psum_hr.  Emitted (and
                # executed) before the first PE t matmul so the two PSUM
                # accumulation groups stay sequential on the PE queue.
                chunks = []
                for row, src in ((0, h_in), (1, r_in)):
                    for c0 in range(0, B_FAST, 128):
                        k = min(128, B_FAST - c0)
                        ht = hrpool.tile([128, D], f32)
                        nc.gpsimd.dma_start(ht[:k, :], src[c0 : c0 + k, :])
                        chunks.append((row, ht, k))
                for i, (row, ht, k) in enumerate(chunks):
                    nc.tensor.matmul(
                        psum_hr[:],
                        W[:k, 128 - row : 256 - row],
                        ht[:k, :],
                        start=(i == 0),
                        stop=(i == len(chunks) - 1),
                    )

            n_pe = sum(1 for it in TILE_PLAN if it[0] == "pe")
            pe_done = 0
            dve_done = 0
            for k, item in enumerate(TILE_PLAN):
                if k == 1:
                    emit_setup_and_hr()
                # Alternate rings WITHIN each path: with the 1:1 D,P plan
                # interleave, k-parity ring choice put every DVE tile on
                # the sync ring and every PE tile on the scalar ring, so
                # the DVE byte stream serialized through one ring and its
                # drain phase ran single-ring (~303 GB/s).  Per-path
                # alternation keeps both rings fed with a mix.
                if item[0] == "pe":
                    dma = nc.scalar if pe_done % 2 == 0 else nc.sync
                else:
                    dma = nc.sync if dve_done % 2 == 0 else nc.scalar
                    dve_done += 1
                tl = loads.tile([128, 16 * MMW], f32)
                if item[0] == "pe":
                    # Batch-major tile: rows [r0, r0+nr) x cols [8192j, +8192).
                    _, r0, nr, j = item
                    dma.dma_start(
                        tl[:nr, :], t_in[r0 : r0 + nr, 8192 * j : 8192 * (j + 1)]
                    )
                    for q in range(16):
                        qg = 16 * j + q  # global column block -> psum row
                        nc.tensor.matmul(
                            psum_t[:],
                            W[:nr, 128 - qg : 256 - qg],
                            tl[:nr, MMW * q : MMW * (q + 1)],
                            start=(pe_done == 0 and q == 0),
                            stop=(pe_done == n_pe - 1 and q == 15),
                        )
                    pe_done += 1
                    continue
                _, b0, NB, cnd = item
                fw = NB * MMW
                src = t_in[b0 : b0 + NB, :].rearrange("b (p c) -> p b c", p=128)
                dst = tl[:, :fw].rearrange("p (b c) -> p b c", b=NB)
                if cnd:
                    # Skipped on the slow core(s): the slot then holds stale
                    # (finite) data; the masked merge zeroes it.
                    dma.dma_start(dst, src, cond=skip_cond[cnd][dma])
                else:
                    dma.dma_start(dst, src)
                half = fw // 2
                while half >= 1024:
                    nc.vector.tensor_tensor(
                        tl[:, :half], tl[:, :half], tl[:, half : 2 * half], add
                    )
                    half //= 2
                if k == 0:
                    nc.vector.tensor_copy(acc[:], tl[:, :1024])
                elif cnd:
                    # acc = (tile_fold * mask) + acc
                    nc.vector.scalar_tensor_tensor(
                        acc[:],
                        tl[:, :1024],
                        masks[cnd][:],
                        acc[:],
                        mybir.AluOpType.mult,
                        add,
                    )
                else:
                    nc.vector.tensor_tensor(acc[:], acc[:], tl[:, :1024], add)

            res_t = res.tile([128, MMW], f32)
            nc.vector.tensor_tensor(res_t[:], acc[:, :512], acc[:, 512:], add)
            nc.vector.tensor_tensor(res_t[:], res_t[:], psum_t[:], add)
            nc.sync.dma_start(out_t[:], res_t[:])

            res_hr = res.tile([2, D], f32)
            nc.vector.tensor_copy(res_hr[:], psum_hr[0:2, :])
            nc.sync.dma_start(out_hr[:], res_hr[:])

    nc.compile()
    return nc


def _get_built():
    global _BUILT
    if _BUILT is None:
        _BUILT = _build()
    return _BUILT


def kernel(h, r, t, w_i, w_j, w_k):
    global LAST_RESULTS
    from concourse import bass_utils

    nc = _get_built()
    t2 = np.ascontiguousarray(t, dtype=np.float32).reshape(B, FLAT)
    h = np.ascontiguousarray(h, dtype=np.float32)
    r = np.ascontiguousarray(r, dtype=np.float32)

    def pad(a, ncols):
        out = np.zeros((B_FAST, ncols), dtype=np.float32)
        out[: a.shape[0]] = a
        return out

    starts = np.concatenate([[0], np.cumsum(SIZES)])
    in_maps = []
    for c in range(NCORES):
        s, e = int(starts[c]), int(starts[c + 1])
        if e - s == B_FAST:
            in_maps.append({"t_shard": t2[s:e], "h_shard": h[s:e], "r_shard": r[s:e]})
        else:
            in_maps.append(
                {
                    "t_shard": pad(t2[s:e], FLAT),
                    "h_shard": pad(h[s:e], D),
                    "r_shard": pad(r[s:e], D),
                }
            )
    results = bass_utils.run_bass_kernel_spmd(
        nc, in_maps, core_ids=list(range(NCORES)), **RUN_KWARGS
    )
    LAST_RESULTS = results

    sum_t = np.zeros(FLAT, dtype=np.float64)
    sum_h = np.zeros(D, dtype=np.float64)
    sum_r = np.zeros(D, dtype=np.float64)
    for c in range(NCORES):
        sum_t += results.results[c]["out_t_part"].reshape(FLAT)
        sum_h += results.results[c]["out_hr_part"][0]
        sum_r += results.results[c]["out_hr_part"][1]

    out = np.empty((N, 3 * D), dtype=np.float32)
    out[:, 0:D] = sum_h.astype(np.float32)[None, :]
    out[:, D : 2 * D] = sum_r.astype(np.float32)[None, :]
    out[:, 2 * D :] = sum_t.astype(np.float32).reshape(N, D)
    return out
